# revision 14
# baseline (speedup 1.0000x reference)
"""Trainium2 Bass kernel for TimeSformer-style divided space attention.

Problem: x[4,3137,1024] -> qkv proj (16 heads, dh=64) -> per-frame spatial
attention (cls token attends globally; each frame's 196 patches attend to
frame + cls) -> out proj.

Sharding: 8 cores = 4 batches x 2 head-groups (8 heads each). Each core
computes a full [3137,1024] partial output (its head-group's contribution
through w_out); host sums the two partials per batch. The cls token's
output row is finalized on the host from tiny exported per-frame cls
numerator/denominator strips (kills the serial device-side tail).

Transpose-free attention: sim is computed directly in [keys, queries]
orientation (stationary kT, moving qT), with the cls query replicated as
column 196 of every frame block so cls attention rides along the per-frame
matmuls. v carries a ones column so the softmax denominator falls out of
the same PE matmul that produces the output (ot rows 0..63 = d, row 64 =
sum of exps). Normalization per head-pair: DVE copy (partition-shift from
PSUM) of both heads' denominators into one strip, one fast-approx
reciprocal, one gpsimd partition broadcast, two DVE multiplies.

Schedule (frame-major): v projection first (25 token-chunks of 128, PE
optimal), then per qk chunk c (2 frames x 8 m-slices): attention of frames
{2c-1, 2c} for all 4 head-pairs + out-proj drain of completed attnT
columns (held back by a small reserve so the PE stays fed through the
final frame's DVE chain). Frame f's sim chunk-b reads 59 columns into
frame f+1's kTx block, hence the one-chunk lag.

Host-side DMA-friendly layouts (each row 1-8KB contiguous):
  xt_d   [128, KD*T]    x[b]^T packed per 128-row contraction chunk
  wq_d   [128, KD*1536] [v | q*scale | k] column order per chunk
  wout_d [128, 4*1024]  row-chunked w_out slice
Device layouts (matmul operands bf16, accumulation fp32):
  qTx/kTx sbuf [128, 4, 16*197(+pad)]: partition = (h%2)*64 + d,
        free = (head-pair, frame-block of [196 tokens | cls])
  v_lin  sbuf [128, 25, 512] token-chunked v (aliases attnT's buffer),
        DMA-rearranged into frame-aligned v_fr [128, 32, 8, 65]
        (cls_v in partition 68 of odd chunks, ones column at 64)
  attnT  sbuf [128, 4, 3137] d-major attention output
"""

import numpy as np
import ml_dtypes

B = 4
T = 3137          # 1 + 16*196
TP = T - 1        # 3136 patch tokens
D = 1024
NH = 8            # heads per core
DH = 64
F = 16
NP = 196
KB = NP + 1       # 197: per-frame block = 196 patches + cls
HD = NH * DH      # 512
QKV = 3 * HD      # 1536
KD = D // 128     # 8 contraction chunks
NC = TP // 128 + 1  # 25 v-projection token chunks (last has 64 rows)
N_CORES = 8
SCALE = DH ** -0.5
KPAD = 59         # zero pad after kTx blocks so chunk-b sims read defined data
RESERVE = 3       # out-proj chunks held back as PE filler for the tail

bf16 = ml_dtypes.bfloat16

_CACHE = {}


def _vfr_pieces():
    """Static (src_chunk, src_row0, dst_fc, dst_row0, nrows) DMA pieces
    mapping v_lin token-chunks onto frame-aligned v_fr."""
    pieces = []
    for fc in range(2 * F):
        f, half = divmod(fc, 2)
        a = NP * f + 128 * half            # patch index (token-1)
        n = 128 if half == 0 else NP - 128
        s = a
        while s < a + n:
            e = min(a + n, (s // 128 + 1) * 128)
            pieces.append((s // 128, s % 128, fc, s - a, e - s))
            s = e
    return pieces


def _build_nc():
    from concourse import bacc, mybir, tile
    from contextlib import ExitStack

    dt = mybir.dt
    AF = mybir.ActivationFunctionType

    nc = bacc.Bacc(None, target_bir_lowering=False, debug=False)

    xt_d = nc.dram_tensor("xt", [128, KD * T], dt.bfloat16,
                          kind="ExternalInput")
    wq_d = nc.dram_tensor("wqkv", [128, KD * QKV], dt.bfloat16,
                          kind="ExternalInput")
    # xt_d: token-block-major pack [blk][k][tw]; wq_d: [v-all|q-all|k-all]
    wout_d = nc.dram_tensor("wout", [128, 4 * D], dt.bfloat16,
                            kind="ExternalInput")
    out_d = nc.dram_tensor("out", [T, D], dt.bfloat16, kind="ExternalOutput")
    cls_d = nc.dram_tensor("cls", [65, NH * F], dt.float32,
                           kind="ExternalOutput")
    vcls_d = nc.dram_tensor("vcls", [1, HD], dt.bfloat16,
                            kind="ExternalOutput")

    with tile.TileContext(nc) as tc, ExitStack() as ctx:
        # ---- static tiles (live for the whole kernel) ----
        stat = ctx.enter_context(tc.tile_pool(name="stat", bufs=1))
        xt_sb = stat.tile([128, KD, T], dt.bfloat16)
        wq_sb = stat.tile([128, KD, QKV], dt.bfloat16)
        wout_sb = stat.tile([128, 4, D], dt.bfloat16)
        qTx = stat.tile([128, 4, F * KB], dt.bfloat16)
        kTx = stat.tile([128, 4, F * KB + KPAD], dt.bfloat16)
        clsqk = stat.tile([128, 8], dt.float32)
        attnT = stat.tile([128, 4, T], dt.bfloat16)
        v_fr = stat.tile([128, 2 * F, NH, DH + 1], dt.bfloat16)
        vcls = stat.tile([1, NH, DH], dt.bfloat16)
        cls_st = stat.tile([128, NH, F], dt.float32)
        ones16 = stat.tile([128, 16], dt.float32)

        nc.vector.memset(ones16[:, :], 1.0)
        nc.vector.memset(kTx[:, :, F * KB:], 0.0)
        nc.vector.memset(v_fr[:, :, :, DH:DH + 1], 1.0)

        with ExitStack() as p1:
            # ---- input DMA: split per k-chunk across queues, dispatched
            # from both HWDGE engines (sync + scalar) to halve the serial
            # dispatch latency at the head ----
            wq_v = wq_d[:, :].rearrange("p (s k q) -> p s k q", s=3, q=HD)
            xt_blk = []          # token-block views of the packed xt dram
            off = 0
            for bi in range(7):
                t0, t1 = 512 * bi, min(512 * (bi + 1), T)
                xt_blk.append((t0, t1, off))
                off += KD * (t1 - t0)

            def xt_dma(eng, k, bi):
                t0, t1, o = xt_blk[bi]
                tw = t1 - t0
                src = xt_d[:, o:o + KD * tw].rearrange(
                    "p (k t) -> p k t", t=tw)
                eng.dma_start(xt_sb[:, k, t0:t1], src[:, k, :])

            for k in range(KD):
                eng = nc.sync if k % 2 == 0 else nc.scalar
                eng.dma_start(wq_sb[:, k, 0:HD], wq_v[:, 0, k, :])
            for k in range(KD):
                xt_dma(nc.sync if k % 2 == 0 else nc.scalar, k, 0)
            for bi in range(1, 7):
                for k in range(KD):
                    xt_dma(nc.sync if k % 2 == 0 else nc.scalar, k, bi)
            for k in range(KD):
                eng = nc.sync if k % 2 == 0 else nc.scalar
                eng.dma_start(wq_sb[:, k, HD:QKV], wq_v[:, 1:3, k, :])
            nc.sync.dma_start(
                wout_sb[:, :, :],
                wout_d[:, :].rearrange("p (c o) -> p c o", o=D))

            # ---- v projection: cls row + frame-aligned token chunks ----
            with ExitStack() as pv:
                ps_v = pv.enter_context(
                    tc.tile_pool(name="ps_v", bufs=3, space="PSUM"))
                ps = ps_v.tile([128, HD], dt.float32, tag="v")
                for k in range(KD):
                    nc.tensor.matmul(ps[:1, :], xt_sb[:, k, 0:1],
                                     wq_sb[:, k, 0:HD],
                                     start=(k == 0), stop=(k == KD - 1))
                nc.vector.tensor_copy(vcls[:, :, :], ps[:1, :])
                for f in range(F):
                    for jc in range(2):
                        r0 = 1 + NP * f + 128 * jc
                        rn = 128 if jc == 0 else NP - 128
                        ps = ps_v.tile([128, HD], dt.float32, tag="v")
                        for k in range(KD):
                            nc.tensor.matmul(
                                ps[:rn, :], xt_sb[:, k, r0:r0 + rn],
                                wq_sb[:, k, 0:HD],
                                start=(k == 0), stop=(k == KD - 1))
                        eng = nc.scalar if jc == 0 else nc.vector
                        if jc == 0:
                            eng.copy(v_fr[:rn, 2 * f, :, 0:DH],
                                     ps[:rn, :].rearrange(
                                         "p (h d) -> p h d", d=DH))
                        else:
                            eng.tensor_copy(v_fr[:rn, 2 * f + 1, :, 0:DH],
                                            ps[:rn, :].rearrange(
                                                "p (h d) -> p h d", d=DH))
            nc.sync.dma_start(vcls_d[:, :], vcls[:, :, :])
            # scatter cls_v into partition 68 of every odd v chunk
            for f in range(F):
                nc.sync.dma_start(v_fr[68:69, 2 * f + 1, :, 0:DH],
                                  vcls[:, :, :])

            ps_gen = p1.enter_context(
                tc.tile_pool(name="ps_gen", bufs=2, space="PSUM"))
            ps_sim = p1.enter_context(
                tc.tile_pool(name="ps_sim", bufs=3, space="PSUM"))
            ps_ot = p1.enter_context(
                tc.tile_pool(name="ps_ot", bufs=3, space="PSUM"))
            sb_p = p1.enter_context(tc.tile_pool(name="sb_p", bufs=5))
            sb_r = p1.enter_context(tc.tile_pool(name="sb_r", bufs=2))
            sb_rb = p1.enter_context(tc.tile_pool(name="sb_rb", bufs=2))
            sb_o = p1.enter_context(tc.tile_pool(name="sb_o", bufs=2))

            qk_views = {}

            def qk_view(m):
                if m not in qk_views:
                    base = (qTx if m < 4 else kTx)[:, m % 4, 0:F * KB]
                    qk_views[m] = base.rearrange("p (f b) -> p f b", b=KB)
                return qk_views[m]

            def qk_chunk(m, c):
                """project q (m<4) / k (m>=4) slice m for frames 2c, 2c+1"""
                dst = qk_view(m)
                wc = HD + 128 * m if m < 4 else 2 * HD + 128 * (m - 4)
                t0 = 0 if c == 0 else 392 * c + 1
                nsz = 393 if c == 0 else 392
                ps = ps_gen.tile([128, 512], dt.float32, tag="g")
                for k in range(KD):
                    nc.tensor.matmul(
                        ps[:, :nsz], wq_sb[:, k, wc:wc + 128],
                        xt_sb[:, k, t0:t0 + nsz],
                        start=(k == 0), stop=(k == KD - 1))
                s0 = 1 if c == 0 else 0
                if c == 0:
                    nc.vector.tensor_copy(clsqk[:, m:m + 1], ps[:, 0:1])
                if m < 4:
                    nc.scalar.copy(dst[:, 2 * c:2 * c + 2, 0:NP],
                                   ps[:, s0:s0 + 2 * NP])
                else:
                    nc.vector.tensor_copy(dst[:, 2 * c:2 * c + 2, 0:NP],
                                          ps[:, s0:s0 + 2 * NP])

            def out_proj(t0, tn, split_dma=False):
                """project attnT cols [t0, t0+tn) through w_out"""
                o_sb = sb_o.tile([128, D], dt.bfloat16, tag="o")
                for half in range(2):
                    ps = ps_gen.tile([128, 512], dt.float32, tag="g")
                    for kc in range(4):
                        nc.tensor.matmul(
                            ps[:tn, :], attnT[:, kc, t0:t0 + tn],
                            wout_sb[:, kc, half * 512:(half + 1) * 512],
                            start=(kc == 0), stop=(kc == 3))
                    if half == 0:
                        nc.scalar.copy(o_sb[:tn, 0:512], ps[:tn, :])
                    else:
                        nc.vector.tensor_copy(o_sb[:tn, 512:1024], ps[:tn, :])
                if split_dma:
                    for rr in range(0, tn, 32):
                        rn = min(32, tn - rr)
                        nc.sync.dma_start(
                            out_d[1 + t0 + rr:1 + t0 + rr + rn, :],
                            o_sb[rr:rr + rn, :])
                else:
                    nc.sync.dma_start(out_d[1 + t0:1 + t0 + tn, :],
                                      o_sb[:tn, :])

            pend = {}

            def attn_sim(pr, f):
                """sim matmuls + exp for heads (2pr, 2pr+1) of frame f"""
                hc = pr
                fr0 = KB * f
                pTs = []
                for i in range(2):
                    pb = 64 * i
                    sim = ps_sim.tile([128, 2, KB], dt.float32, tag="sim")
                    nc.tensor.matmul(
                        sim[:, 0, :], kTx[pb:pb + 64, hc, fr0:fr0 + 128],
                        qTx[pb:pb + 64, hc, fr0:fr0 + KB],
                        start=True, stop=True)
                    nc.tensor.matmul(
                        sim[:, 1, :], kTx[pb:pb + 64, hc, fr0 + 128:fr0 + 256],
                        qTx[pb:pb + 64, hc, fr0:fr0 + KB],
                        start=True, stop=True)
                    pT = sb_p.tile([128, 2, KB], dt.bfloat16, tag="p")
                    nc.scalar.activation(pT[:, :, :], sim[:, :, :], AF.Exp)
                    pTs.append(pT)
                pend[(pr, f)] = pTs

            def attn_fin(pr, f):
                """ot matmuls + normalization for heads (2pr, 2pr+1)"""
                hc = pr
                pTs = pend.pop((pr, f))
                ots = []
                for i in range(2):
                    h = 2 * pr + i
                    pT = pTs[i]
                    # output + denominator via the v ones-column: rows 0..63
                    # = d, row 64 = sum(exp)
                    ot = ps_ot.tile([128, KB], dt.float32, tag="ot")
                    nc.tensor.matmul(
                        ot[0:65, :], v_fr[:, 2 * f, h, 0:DH + 1], pT[:, 0, :],
                        start=True, stop=False)
                    nc.tensor.matmul(
                        ot[0:65, :], v_fr[0:69, 2 * f + 1, h, 0:DH + 1],
                        pT[0:69, 1, :], start=False, stop=True)
                    # unnormalized cls numerator (+den at row 64) per frame;
                    # finalized on host
                    nc.scalar.copy(cls_st[0:65, h, f:f + 1], ot[0:65, NP:KB])
                    ots.append(ot)
                # normalize both heads: custom-DVE recip can't
                # partition-shift from PSUM, so stage via plain copies
                r2 = sb_r.tile([1, 2, NP], dt.float32, tag="r")
                r_t = sb_r.tile([1, 2, NP], dt.float32, tag="r2")
                rb = sb_rb.tile([128, 2, NP], dt.float32, tag="rb")
                for i in range(2):
                    nc.vector.tensor_copy(r2[0:1, i, :], ots[i][64:65, 0:NP])
                nc.vector.reciprocal_approx_fast(r_t[0:1, :, :], r2[0:1, :, :])
                nc.gpsimd.partition_broadcast(rb[:, :, :], r_t[0:1, :, :])
                for i in range(2):
                    dlo = 64 * i
                    nc.vector.tensor_mul(
                        attnT[dlo:dlo + 64, hc, NP * f:NP * (f + 1)],
                        ots[i][0:64, 0:NP], rb[dlo:dlo + 64, i, :])

            next_chunk = [0]

            def drain_out_proj(done_cols, reserve=0, limit=99):
                n = 0
                while ((next_chunk[0] + 1 + reserve) * 128 <= done_cols
                       and n < limit):
                    out_proj(128 * next_chunk[0], 128)
                    next_chunk[0] += 1
                    n += 1

            # ---- frame-major main loop ----
            # fine interleave: each attn unit (whose ot/normalize chains run
            # on ACT/DVE/gpsimd) is sandwiched between qk-chunk or out-proj
            # matmul groups so the in-order PE queue always has independent
            # work while the serial engines catch up
            for c in range(8):
                if c == 0:
                    for m in range(8):
                        qk_chunk(m, c)
                    # replicate cls q/k into column 196 of every frame block
                    # (cls lives in chunk 0, so one batched op per m)
                    for m in range(8):
                        nc.vector.tensor_scalar_mul(
                            qk_view(m)[:, :, NP], ones16[:, 0:F],
                            clsqk[:, m:m + 1])
                    for pr in range(4):
                        attn_sim(pr, 0)
                    for pr in range(4):
                        attn_fin(pr, 0)
                else:
                    fa, fb = 2 * c - 1, 2 * c
                    # qk slices for pair pr land right before pr's sims
                    for pr in range(4):
                        qk_chunk(pr, c)
                        qk_chunk(pr + 4, c)
                        attn_sim(pr, fa)
                        if pr == 1:
                            drain_out_proj(NP * fa, reserve=RESERVE, limit=1)
                        if pr >= 2:
                            attn_fin(pr - 2, fa)
                            attn_sim(pr - 2, fb)
                    attn_fin(2, fa)
                    attn_sim(2, fb)
                    drain_out_proj(NP * fa, reserve=RESERVE, limit=1)
                    attn_fin(3, fa)
                    attn_sim(3, fb)
                    attn_fin(0, fb)
                    drain_out_proj(NP * fa, reserve=RESERVE, limit=1)
                    attn_fin(1, fb)
                    attn_fin(2, fb)
                    drain_out_proj(NP * fa, reserve=RESERVE, limit=1)
                    attn_fin(3, fb)
                    drain_out_proj(NP * (fb + 1), reserve=RESERVE)
            # frame 15 interleaved with the reserved drain chunks
            attn_sim(0, 15)
            attn_sim(1, 15)
            drain_out_proj(NP * 15, reserve=2)
            attn_sim(2, 15)
            attn_sim(3, 15)
            drain_out_proj(NP * 15, reserve=1)
            attn_fin(0, 15)
            drain_out_proj(NP * 15, reserve=0)
            attn_fin(1, 15)
            attn_fin(2, 15)
            attn_fin(3, 15)
            drain_out_proj(NP * F)
            out_proj(TP // 128 * 128, TP - TP // 128 * 128, split_dma=True)
            # tiny cls exports; the cls output row is finalized on host
            nc.sync.dma_start(cls_d[:, :], cls_st[0:65, :, :])

    nc.compile()
    return nc


def _get_nc():
    if "nc" not in _CACHE:
        _CACHE["nc"] = _build_nc()
    return _CACHE["nc"]


def _pack_rows(a, kd):
    """[kd*128, N] -> [128, kd*N] with row p = concat_k a[128k+p]"""
    n = a.shape[1]
    return np.ascontiguousarray(
        a.reshape(kd, 128, n).transpose(1, 0, 2).reshape(128, kd * n))


def _prep_in_maps(x, w_qkv, w_out):
    x = np.asarray(x, dtype=np.float32)
    w_qkv = np.asarray(w_qkv, dtype=np.float32)
    w_out = np.asarray(w_out, dtype=np.float32)
    in_maps = []
    for core in range(N_CORES):
        b, hg = divmod(core, 2)
        cs = slice(hg * HD, (hg + 1) * HD)
        # wq_d: [v-all-k | q-all-k | k-all-k], each part [128, KD, 512]
        parts = [w_qkv[:, 2 * D:][:, cs], w_qkv[:, 0 * D:][:, cs] * SCALE,
                 w_qkv[:, 1 * D:][:, cs]]
        wq = np.concatenate([_pack_rows(p, KD) for p in parts], axis=1)
        # xt_d: token-block-major pack [blk][k][tw]
        xtr = x[b].T.reshape(KD, 128, T).transpose(1, 0, 2)
        blks = [xtr[:, :, 512 * i:min(512 * (i + 1), T)].reshape(128, -1)
                for i in range(7)]
        xt = np.concatenate(blks, axis=1)
        wo = w_out[hg * HD:(hg + 1) * HD, :]
        in_maps.append({
            "xt": np.ascontiguousarray(xt).astype(bf16),
            "wqkv": np.ascontiguousarray(wq).astype(bf16),
            "wout": _pack_rows(wo, 4).astype(bf16),
        })
    return in_maps


def _host_cls_row(res_core, w_out, hg, e0):
    """cls output row partial for one core from exported strips (fp32)."""
    cs = np.asarray(res_core["cls"], np.float32).reshape(65, NH, F)
    vq = np.asarray(res_core["vcls"], np.float32).reshape(NH, DH)
    num = cs[0:64].sum(-1)                       # [64, NH]
    den = cs[64].sum(-1)                         # [NH]
    num = num - (F - 1) * e0[None, :] * vq.T     # over-counted cls key
    den = den - (F - 1) * e0
    attnrow = (num / den[None, :]).T.reshape(HD)  # [NH*DH]
    return attnrow @ w_out[hg * HD:(hg + 1) * HD, :]


def run(x, w_qkv, w_out, trace=False):
    from concourse.bass_utils import run_bass_kernel_spmd

    nc = _get_nc()
    in_maps = _prep_in_maps(x, w_qkv, w_out)
    res = run_bass_kernel_spmd(nc, in_maps, list(range(N_CORES)), trace=trace)
    w_out32 = np.asarray(w_out, np.float32)
    # e0 = exp(q_cls . k_cls) per (batch, head), bf16-rounded like the device
    xb = np.asarray(x, np.float32)[:, 0, :].astype(bf16).astype(np.float32)
    wq8 = (np.asarray(w_qkv, np.float32)[:, 0:D] * SCALE
           ).astype(bf16).astype(np.float32)
    wk8 = np.asarray(w_qkv, np.float32)[:, D:2 * D
                                        ].astype(bf16).astype(np.float32)
    qc = (xb @ wq8).astype(bf16).astype(np.float32).reshape(B, 2 * NH, DH)
    kc = (xb @ wk8).astype(bf16).astype(np.float32).reshape(B, 2 * NH, DH)
    e0 = np.exp((qc * kc).sum(-1))                      # [B, 16]
    out = np.empty((B, T, D), dtype=np.float32)
    for b in range(B):
        out[b] = (np.asarray(res.results[2 * b]["out"], np.float32)
                  + np.asarray(res.results[2 * b + 1]["out"], np.float32))
        out[b, 0] = (
            _host_cls_row(res.results[2 * b], w_out32, 0, e0[b, 0:NH])
            + _host_cls_row(res.results[2 * b + 1], w_out32, 1, e0[b, NH:]))
    return out, res


def kernel(x, w_qkv, w_out, f):
    assert int(f) == F
    out, _ = run(x, w_qkv, w_out, trace=False)
    return out


# revision 16
# speedup vs baseline: 1.0250x; 1.0250x over previous
"""Trainium2 Bass kernel for TimeSformer-style divided space attention.

Problem: x[4,3137,1024] -> qkv proj (16 heads, dh=64) -> per-frame spatial
attention (cls token attends globally; each frame's 196 patches attend to
frame + cls) -> out proj.

Sharding: 8 cores = 4 batches x 2 head-groups (8 heads each). Each core
computes a full [3137,1024] partial output (its head-group's contribution
through w_out); host sums the two partials per batch. The cls token's
output row is finalized on the host from tiny exported per-frame cls
numerator/denominator strips (kills the serial device-side tail).

Transpose-free attention: sim is computed directly in [keys, queries]
orientation (stationary kT, moving qT), with the cls query replicated as
column 196 of every frame block so cls attention rides along the per-frame
matmuls. v carries a ones column so the softmax denominator falls out of
the same PE matmul that produces the output (ot rows 0..63 = d, row 64 =
sum of exps). Normalization per head-pair: DVE copy (partition-shift from
PSUM) of both heads' denominators into one strip, one fast-approx
reciprocal, one gpsimd partition broadcast, two DVE multiplies.

Schedule (frame-major): v projection first (25 token-chunks of 128, PE
optimal), then per qk chunk c (2 frames x 8 m-slices): attention of frames
{2c-1, 2c} for all 4 head-pairs + out-proj drain of completed attnT
columns (held back by a small reserve so the PE stays fed through the
final frame's DVE chain). Frame f's sim chunk-b reads 59 columns into
frame f+1's kTx block, hence the one-chunk lag.

Host-side DMA-friendly layouts (each row 1-8KB contiguous):
  xt_d   [128, KD*T]    x[b]^T packed per 128-row contraction chunk
  wq_d   [128, KD*1536] [v | q*scale | k] column order per chunk
  wout_d [128, 4*1024]  row-chunked w_out slice
Device layouts (matmul operands bf16, accumulation fp32):
  qTx/kTx sbuf [128, 4, 16*197(+pad)]: partition = (h%2)*64 + d,
        free = (head-pair, frame-block of [196 tokens | cls])
  v_lin  sbuf [128, 25, 512] token-chunked v (aliases attnT's buffer),
        DMA-rearranged into frame-aligned v_fr [128, 32, 8, 65]
        (cls_v in partition 68 of odd chunks, ones column at 64)
  attnT  sbuf [128, 4, 3137] d-major attention output
"""

import numpy as np
import ml_dtypes

B = 4
T = 3137          # 1 + 16*196
TP = T - 1        # 3136 patch tokens
D = 1024
NH = 8            # heads per core
DH = 64
F = 16
NP = 196
KB = NP + 1       # 197: per-frame block = 196 patches + cls
HD = NH * DH      # 512
QKV = 3 * HD      # 1536
KD = D // 128     # 8 contraction chunks
NC = TP // 128 + 1  # 25 v-projection token chunks (last has 64 rows)
N_CORES = 8
SCALE = DH ** -0.5
KPAD = 59         # zero pad after kTx blocks so chunk-b sims read defined data
RESERVE = 3       # out-proj chunks held back as PE filler for the tail

bf16 = ml_dtypes.bfloat16

_CACHE = {}


def _vfr_pieces():
    """Static (src_chunk, src_row0, dst_fc, dst_row0, nrows) DMA pieces
    mapping v_lin token-chunks onto frame-aligned v_fr."""
    pieces = []
    for fc in range(2 * F):
        f, half = divmod(fc, 2)
        a = NP * f + 128 * half            # patch index (token-1)
        n = 128 if half == 0 else NP - 128
        s = a
        while s < a + n:
            e = min(a + n, (s // 128 + 1) * 128)
            pieces.append((s // 128, s % 128, fc, s - a, e - s))
            s = e
    return pieces


def _build_nc():
    from concourse import bacc, mybir, tile
    from contextlib import ExitStack

    dt = mybir.dt
    AF = mybir.ActivationFunctionType

    nc = bacc.Bacc(None, target_bir_lowering=False, debug=False)

    xt_d = nc.dram_tensor("xt", [128, KD * T], dt.bfloat16,
                          kind="ExternalInput")
    wq_d = nc.dram_tensor("wqkv", [128, KD * QKV], dt.bfloat16,
                          kind="ExternalInput")
    # xt_d: k-chunk-major pack; wq_d: [v-all | q-all | k-all]
    wout_d = nc.dram_tensor("wout", [128, 4 * D], dt.bfloat16,
                            kind="ExternalInput")
    out_d = nc.dram_tensor("out", [T, D], dt.bfloat16, kind="ExternalOutput")
    cls_d = nc.dram_tensor("cls", [65, NH * F], dt.float32,
                           kind="ExternalOutput")
    vcls_d = nc.dram_tensor("vcls", [1, HD], dt.bfloat16,
                            kind="ExternalOutput")

    with tile.TileContext(nc) as tc, ExitStack() as ctx:
        # ---- static tiles (live for the whole kernel) ----
        stat = ctx.enter_context(tc.tile_pool(name="stat", bufs=1))
        xt_sb = stat.tile([128, KD, T], dt.bfloat16)
        wq_sb = stat.tile([128, KD, QKV], dt.bfloat16)
        wout_sb = stat.tile([128, 4, D], dt.bfloat16)
        qTx = stat.tile([128, 4, F * KB], dt.bfloat16)
        kTx = stat.tile([128, 4, F * KB + KPAD], dt.bfloat16)
        clsqk = stat.tile([128, 8], dt.float32)
        attnT = stat.tile([128, 4, T], dt.bfloat16)
        v_fr = stat.tile([128, 2 * F, NH, DH + 1], dt.bfloat16)
        vcls = stat.tile([1, NH, DH], dt.bfloat16)
        cls_st = stat.tile([128, NH, F], dt.float32)
        ones16 = stat.tile([128, 16], dt.float32)

        nc.vector.memset(ones16[:, :], 1.0)
        nc.vector.memset(kTx[:, :, F * KB:], 0.0)
        nc.vector.memset(v_fr[:, :, :, DH:DH + 1], 1.0)

        with ExitStack() as p1:
            # ---- input DMA ----
            # per-queue bandwidth is ~22GB/s, so parallelism needs many
            # queues; but each dispatch costs ~0.7-1us on its engine, so
            # spread dispatches over all three DMA-capable engines and size
            # transfers so arrival order matches the consumption order
            # (4 frame-aligned token waves x 8 contraction chunks)
            wq_v = wq_d[:, :].rearrange("p (s k q) -> p s k q", s=3, q=HD)
            dma_engs = [nc.sync, nc.scalar, nc.gpsimd]
            di = [0]

            def in_dma(dst, src):
                dma_engs[di[0] % 3].dma_start(dst, src)
                di[0] += 1

            for k in range(KD):
                in_dma(wq_sb[:, k, 0:HD], wq_v[:, 0, k, :])
            xt_v = xt_d[:, :].rearrange("p (k t) -> p k t", t=T)
            for r in range(4):
                t0 = 0 if r == 0 else 1 + 784 * r
                t1 = 1 + 784 * (r + 1)
                for k in range(KD):
                    in_dma(xt_sb[:, k, t0:t1], xt_v[:, k, t0:t1])
            for k in range(KD):
                in_dma(wq_sb[:, k, HD:QKV], wq_v[:, 1:3, k, :])
            nc.sync.dma_start(
                wout_sb[:, 0:2, :],
                wout_d[:, 0:2 * D].rearrange("p (c o) -> p c o", o=D))
            nc.scalar.dma_start(
                wout_sb[:, 2:4, :],
                wout_d[:, 2 * D:].rearrange("p (c o) -> p c o", o=D))

            # ---- v projection: cls row + frame-aligned token chunks ----
            with ExitStack() as pv:
                ps_v = pv.enter_context(
                    tc.tile_pool(name="ps_v", bufs=3, space="PSUM"))
                ps = ps_v.tile([128, HD], dt.float32, tag="v")
                for k in range(KD):
                    nc.tensor.matmul(ps[:1, :], xt_sb[:, k, 0:1],
                                     wq_sb[:, k, 0:HD],
                                     start=(k == 0), stop=(k == KD - 1))
                nc.vector.tensor_copy(vcls[:, :, :], ps[:1, :])
                for f in range(F):
                    for jc in range(2):
                        r0 = 1 + NP * f + 128 * jc
                        rn = 128 if jc == 0 else NP - 128
                        ps = ps_v.tile([128, HD], dt.float32, tag="v")
                        for k in range(KD):
                            nc.tensor.matmul(
                                ps[:rn, :], xt_sb[:, k, r0:r0 + rn],
                                wq_sb[:, k, 0:HD],
                                start=(k == 0), stop=(k == KD - 1))
                        eng = nc.scalar if jc == 0 else nc.vector
                        if jc == 0:
                            eng.copy(v_fr[:rn, 2 * f, :, 0:DH],
                                     ps[:rn, :].rearrange(
                                         "p (h d) -> p h d", d=DH))
                        else:
                            eng.tensor_copy(v_fr[:rn, 2 * f + 1, :, 0:DH],
                                            ps[:rn, :].rearrange(
                                                "p (h d) -> p h d", d=DH))
            nc.sync.dma_start(vcls_d[:, :], vcls[:, :, :])
            # scatter cls_v into partition 68 of every odd v chunk
            for f in range(F):
                nc.sync.dma_start(v_fr[68:69, 2 * f + 1, :, 0:DH],
                                  vcls[:, :, :])

            ps_gen = p1.enter_context(
                tc.tile_pool(name="ps_gen", bufs=2, space="PSUM"))
            ps_sim = p1.enter_context(
                tc.tile_pool(name="ps_sim", bufs=3, space="PSUM"))
            ps_ot = p1.enter_context(
                tc.tile_pool(name="ps_ot", bufs=3, space="PSUM"))
            sb_p = p1.enter_context(tc.tile_pool(name="sb_p", bufs=5))
            sb_r = p1.enter_context(tc.tile_pool(name="sb_r", bufs=2))
            sb_rb = p1.enter_context(tc.tile_pool(name="sb_rb", bufs=2))
            sb_o = p1.enter_context(tc.tile_pool(name="sb_o", bufs=2))

            qk_views = {}

            def qk_view(m):
                if m not in qk_views:
                    base = (qTx if m < 4 else kTx)[:, m % 4, 0:F * KB]
                    qk_views[m] = base.rearrange("p (f b) -> p f b", b=KB)
                return qk_views[m]

            def qk_chunk(m, c):
                """project q (m<4) / k (m>=4) slice m for frames 2c, 2c+1"""
                dst = qk_view(m)
                wc = HD + 128 * m if m < 4 else 2 * HD + 128 * (m - 4)
                t0 = 0 if c == 0 else 392 * c + 1
                nsz = 393 if c == 0 else 392
                ps = ps_gen.tile([128, 512], dt.float32, tag="g")
                for k in range(KD):
                    nc.tensor.matmul(
                        ps[:, :nsz], wq_sb[:, k, wc:wc + 128],
                        xt_sb[:, k, t0:t0 + nsz],
                        start=(k == 0), stop=(k == KD - 1))
                s0 = 1 if c == 0 else 0
                if c == 0:
                    nc.vector.tensor_copy(clsqk[:, m:m + 1], ps[:, 0:1])
                if m < 4:
                    nc.scalar.copy(dst[:, 2 * c:2 * c + 2, 0:NP],
                                   ps[:, s0:s0 + 2 * NP])
                else:
                    nc.vector.tensor_copy(dst[:, 2 * c:2 * c + 2, 0:NP],
                                          ps[:, s0:s0 + 2 * NP])

            def out_proj(t0, tn, split_dma=False):
                """project attnT cols [t0, t0+tn) through w_out"""
                o_sb = sb_o.tile([128, D], dt.bfloat16, tag="o")
                for half in range(2):
                    ps = ps_gen.tile([128, 512], dt.float32, tag="g")
                    for kc in range(4):
                        nc.tensor.matmul(
                            ps[:tn, :], attnT[:, kc, t0:t0 + tn],
                            wout_sb[:, kc, half * 512:(half + 1) * 512],
                            start=(kc == 0), stop=(kc == 3))
                    if half == 0:
                        nc.scalar.copy(o_sb[:tn, 0:512], ps[:tn, :])
                    else:
                        nc.vector.tensor_copy(o_sb[:tn, 512:1024], ps[:tn, :])
                if split_dma:
                    for rr in range(0, tn, 32):
                        rn = min(32, tn - rr)
                        nc.sync.dma_start(
                            out_d[1 + t0 + rr:1 + t0 + rr + rn, :],
                            o_sb[rr:rr + rn, :])
                else:
                    nc.sync.dma_start(out_d[1 + t0:1 + t0 + tn, :],
                                      o_sb[:tn, :])

            pend = {}

            def attn_sim(pr, f):
                """sim matmuls + exp for heads (2pr, 2pr+1) of frame f"""
                hc = pr
                fr0 = KB * f
                pTs = []
                for i in range(2):
                    pb = 64 * i
                    sim = ps_sim.tile([128, 2, KB], dt.float32, tag="sim")
                    nc.tensor.matmul(
                        sim[:, 0, :], kTx[pb:pb + 64, hc, fr0:fr0 + 128],
                        qTx[pb:pb + 64, hc, fr0:fr0 + KB],
                        start=True, stop=True)
                    nc.tensor.matmul(
                        sim[:, 1, :], kTx[pb:pb + 64, hc, fr0 + 128:fr0 + 256],
                        qTx[pb:pb + 64, hc, fr0:fr0 + KB],
                        start=True, stop=True)
                    pT = sb_p.tile([128, 2, KB], dt.bfloat16, tag="p")
                    nc.scalar.activation(pT[:, :, :], sim[:, :, :], AF.Exp)
                    pTs.append(pT)
                pend[(pr, f)] = pTs

            def attn_fin(pr, f):
                """ot matmuls + normalization for heads (2pr, 2pr+1)"""
                hc = pr
                pTs = pend.pop((pr, f))
                ots = []
                for i in range(2):
                    h = 2 * pr + i
                    pT = pTs[i]
                    # output + denominator via the v ones-column: rows 0..63
                    # = d, row 64 = sum(exp)
                    ot = ps_ot.tile([128, KB], dt.float32, tag="ot")
                    nc.tensor.matmul(
                        ot[0:65, :], v_fr[:, 2 * f, h, 0:DH + 1], pT[:, 0, :],
                        start=True, stop=False)
                    nc.tensor.matmul(
                        ot[0:65, :], v_fr[0:69, 2 * f + 1, h, 0:DH + 1],
                        pT[0:69, 1, :], start=False, stop=True)
                    # unnormalized cls numerator (+den at row 64) per frame;
                    # finalized on host
                    nc.scalar.copy(cls_st[0:65, h, f:f + 1], ot[0:65, NP:KB])
                    ots.append(ot)
                # normalize both heads: custom-DVE recip can't
                # partition-shift from PSUM, so stage via plain copies
                r2 = sb_r.tile([1, 2, NP], dt.float32, tag="r")
                r_t = sb_r.tile([1, 2, NP], dt.float32, tag="r2")
                rb = sb_rb.tile([128, 2, NP], dt.float32, tag="rb")
                for i in range(2):
                    nc.vector.tensor_copy(r2[0:1, i, :], ots[i][64:65, 0:NP])
                nc.vector.reciprocal_approx_fast(r_t[0:1, :, :], r2[0:1, :, :])
                nc.gpsimd.partition_broadcast(rb[:, :, :], r_t[0:1, :, :])
                for i in range(2):
                    dlo = 64 * i
                    nc.vector.tensor_mul(
                        attnT[dlo:dlo + 64, hc, NP * f:NP * (f + 1)],
                        ots[i][0:64, 0:NP], rb[dlo:dlo + 64, i, :])

            next_chunk = [0]

            def drain_out_proj(done_cols, reserve=0, limit=99):
                n = 0
                while ((next_chunk[0] + 1 + reserve) * 128 <= done_cols
                       and n < limit):
                    out_proj(128 * next_chunk[0], 128)
                    next_chunk[0] += 1
                    n += 1

            # ---- frame-major main loop ----
            # fine interleave: each attn unit (whose ot/normalize chains run
            # on ACT/DVE/gpsimd) is sandwiched between qk-chunk or out-proj
            # matmul groups so the in-order PE queue always has independent
            # work while the serial engines catch up
            for c in range(8):
                if c == 0:
                    for m in range(8):
                        qk_chunk(m, c)
                    # replicate cls q/k into column 196 of every frame block
                    # (cls lives in chunk 0, so one batched op per m)
                    for m in range(8):
                        nc.vector.tensor_scalar_mul(
                            qk_view(m)[:, :, NP], ones16[:, 0:F],
                            clsqk[:, m:m + 1])
                    for pr in range(4):
                        attn_sim(pr, 0)
                    for pr in range(4):
                        attn_fin(pr, 0)
                else:
                    fa, fb = 2 * c - 1, 2 * c
                    # qk slices for pair pr land right before pr's sims
                    for pr in range(4):
                        qk_chunk(pr, c)
                        qk_chunk(pr + 4, c)
                        attn_sim(pr, fa)
                        if pr == 1:
                            drain_out_proj(NP * fa, reserve=RESERVE, limit=1)
                        if pr >= 2:
                            attn_fin(pr - 2, fa)
                            attn_sim(pr - 2, fb)
                    attn_fin(2, fa)
                    attn_sim(2, fb)
                    drain_out_proj(NP * fa, reserve=RESERVE, limit=1)
                    attn_fin(3, fa)
                    attn_sim(3, fb)
                    attn_fin(0, fb)
                    drain_out_proj(NP * fa, reserve=RESERVE, limit=1)
                    attn_fin(1, fb)
                    attn_fin(2, fb)
                    drain_out_proj(NP * fa, reserve=RESERVE, limit=1)
                    attn_fin(3, fb)
                    drain_out_proj(NP * (fb + 1), reserve=RESERVE)
            # frame 15 interleaved with the reserved drain chunks
            attn_sim(0, 15)
            attn_sim(1, 15)
            drain_out_proj(NP * 15, reserve=2)
            attn_sim(2, 15)
            attn_sim(3, 15)
            drain_out_proj(NP * 15, reserve=1)
            attn_fin(0, 15)
            drain_out_proj(NP * 15, reserve=0)
            attn_fin(1, 15)
            attn_fin(2, 15)
            attn_fin(3, 15)
            drain_out_proj(NP * F)
            out_proj(TP // 128 * 128, TP - TP // 128 * 128, split_dma=True)
            # tiny cls exports; the cls output row is finalized on host
            nc.sync.dma_start(cls_d[:, :], cls_st[0:65, :, :])

    nc.compile()
    return nc


def _get_nc():
    if "nc" not in _CACHE:
        _CACHE["nc"] = _build_nc()
    return _CACHE["nc"]


def _pack_rows(a, kd):
    """[kd*128, N] -> [128, kd*N] with row p = concat_k a[128k+p]"""
    n = a.shape[1]
    return np.ascontiguousarray(
        a.reshape(kd, 128, n).transpose(1, 0, 2).reshape(128, kd * n))


def _prep_in_maps(x, w_qkv, w_out):
    x = np.asarray(x, dtype=np.float32)
    w_qkv = np.asarray(w_qkv, dtype=np.float32)
    w_out = np.asarray(w_out, dtype=np.float32)
    in_maps = []
    for core in range(N_CORES):
        b, hg = divmod(core, 2)
        cs = slice(hg * HD, (hg + 1) * HD)
        # wq_d: [v-all-k | q-all-k | k-all-k], each part [128, KD, 512]
        parts = [w_qkv[:, 2 * D:][:, cs], w_qkv[:, 0 * D:][:, cs] * SCALE,
                 w_qkv[:, 1 * D:][:, cs]]
        wq = np.concatenate([_pack_rows(p, KD) for p in parts], axis=1)
        wo = w_out[hg * HD:(hg + 1) * HD, :]
        in_maps.append({
            "xt": _pack_rows(np.ascontiguousarray(x[b].T), KD).astype(bf16),
            "wqkv": np.ascontiguousarray(wq).astype(bf16),
            "wout": _pack_rows(wo, 4).astype(bf16),
        })
    return in_maps


def _host_cls_row(res_core, w_out, hg, e0):
    """cls output row partial for one core from exported strips (fp32)."""
    cs = np.asarray(res_core["cls"], np.float32).reshape(65, NH, F)
    vq = np.asarray(res_core["vcls"], np.float32).reshape(NH, DH)
    num = cs[0:64].sum(-1)                       # [64, NH]
    den = cs[64].sum(-1)                         # [NH]
    num = num - (F - 1) * e0[None, :] * vq.T     # over-counted cls key
    den = den - (F - 1) * e0
    attnrow = (num / den[None, :]).T.reshape(HD)  # [NH*DH]
    return attnrow @ w_out[hg * HD:(hg + 1) * HD, :]


def run(x, w_qkv, w_out, trace=False):
    from concourse.bass_utils import run_bass_kernel_spmd

    nc = _get_nc()
    in_maps = _prep_in_maps(x, w_qkv, w_out)
    res = run_bass_kernel_spmd(nc, in_maps, list(range(N_CORES)), trace=trace)
    w_out32 = np.asarray(w_out, np.float32)
    # e0 = exp(q_cls . k_cls) per (batch, head), bf16-rounded like the device
    xb = np.asarray(x, np.float32)[:, 0, :].astype(bf16).astype(np.float32)
    wq8 = (np.asarray(w_qkv, np.float32)[:, 0:D] * SCALE
           ).astype(bf16).astype(np.float32)
    wk8 = np.asarray(w_qkv, np.float32)[:, D:2 * D
                                        ].astype(bf16).astype(np.float32)
    qc = (xb @ wq8).astype(bf16).astype(np.float32).reshape(B, 2 * NH, DH)
    kc = (xb @ wk8).astype(bf16).astype(np.float32).reshape(B, 2 * NH, DH)
    e0 = np.exp((qc * kc).sum(-1))                      # [B, 16]
    out = np.empty((B, T, D), dtype=np.float32)
    for b in range(B):
        out[b] = (np.asarray(res.results[2 * b]["out"], np.float32)
                  + np.asarray(res.results[2 * b + 1]["out"], np.float32))
        out[b, 0] = (
            _host_cls_row(res.results[2 * b], w_out32, 0, e0[b, 0:NH])
            + _host_cls_row(res.results[2 * b + 1], w_out32, 1, e0[b, NH:]))
    return out, res


def kernel(x, w_qkv, w_out, f):
    assert int(f) == F
    out, _ = run(x, w_qkv, w_out, trace=False)
    return out


# revision 20
# speedup vs baseline: 1.1261x; 1.0986x over previous
"""Trainium2 Bass kernel for TimeSformer-style divided space attention.

Problem: x[4,3137,1024] -> qkv proj (16 heads, dh=64) -> per-frame spatial
attention (cls token attends globally; each frame's 196 patches attend to
frame + cls) -> out proj.

Sharding: 8 cores = 4 batches x 2 head-groups (8 heads each). Each core
computes a full [3137,1024] partial output (its head-group's contribution
through w_out); host sums the two partials per batch. The cls token's
output row is finalized on the host from tiny exported per-frame cls
numerator/denominator strips (kills the serial device-side tail).

Transpose-free attention: sim is computed directly in [keys, queries]
orientation (stationary kT, moving qT), with the cls query replicated as
column 196 of every frame block so cls attention rides along the per-frame
matmuls. v carries a ones column so the softmax denominator falls out of
the same PE matmul that produces the output (ot rows 0..63 = d, row 64 =
sum of exps). Normalization per head-pair: DVE copy (partition-shift from
PSUM) of both heads' denominators into one strip, one fast-approx
reciprocal, one gpsimd partition broadcast, two DVE multiplies.

Schedule (frame-major): v projection first (25 token-chunks of 128, PE
optimal), then per qk chunk c (2 frames x 8 m-slices): attention of frames
{2c-1, 2c} for all 4 head-pairs + out-proj drain of completed attnT
columns (held back by a small reserve so the PE stays fed through the
final frame's DVE chain). Frame f's sim chunk-b reads 59 columns into
frame f+1's kTx block, hence the one-chunk lag.

Host-side DMA-friendly layouts (each row 1-8KB contiguous):
  xt_d   [128, KD*T]    x[b]^T packed per 128-row contraction chunk
  wq_d   [128, KD*1536] [v | q*scale | k] column order per chunk
  wout_d [128, 4*1024]  row-chunked w_out slice
Device layouts (matmul operands bf16, accumulation fp32):
  qTx/kTx sbuf [128, 4, 16*197(+pad)]: partition = (h%2)*64 + d,
        free = (head-pair, frame-block of [196 tokens | cls])
  v_lin  sbuf [128, 25, 512] token-chunked v (aliases attnT's buffer),
        DMA-rearranged into frame-aligned v_fr [128, 32, 8, 65]
        (cls_v in partition 68 of odd chunks, ones column at 64)
  attnT  sbuf [128, 4, 3137] d-major attention output
"""

import numpy as np
import ml_dtypes

B = 4
T = 3137          # 1 + 16*196
TP = T - 1        # 3136 patch tokens
D = 1024
NH = 8            # heads per core
DH = 64
F = 16
NP = 196
KB = NP + 1       # 197: per-frame block = 196 patches + cls
HD = NH * DH      # 512
QKV = 3 * HD      # 1536
KD = D // 128     # 8 contraction chunks
NC = TP // 128 + 1  # 25 v-projection token chunks (last has 64 rows)
N_CORES = 8
SCALE = DH ** -0.5
KPAD = 59         # zero pad after kTx blocks so chunk-b sims read defined data
RESERVE = 3       # out-proj chunks held back as PE filler for the tail

bf16 = ml_dtypes.bfloat16

_CACHE = {}


def _vfr_pieces():
    """Static (src_chunk, src_row0, dst_fc, dst_row0, nrows) DMA pieces
    mapping v_lin token-chunks onto frame-aligned v_fr."""
    pieces = []
    for fc in range(2 * F):
        f, half = divmod(fc, 2)
        a = NP * f + 128 * half            # patch index (token-1)
        n = 128 if half == 0 else NP - 128
        s = a
        while s < a + n:
            e = min(a + n, (s // 128 + 1) * 128)
            pieces.append((s // 128, s % 128, fc, s - a, e - s))
            s = e
    return pieces


def _build_nc():
    from concourse import bacc, mybir, tile
    from contextlib import ExitStack

    dt = mybir.dt
    AF = mybir.ActivationFunctionType

    nc = bacc.Bacc(None, target_bir_lowering=False, debug=False)

    xt_d = nc.dram_tensor("xt", [128, KD * T], dt.bfloat16,
                          kind="ExternalInput")
    wq_d = nc.dram_tensor("wqkv", [128, KD * QKV], dt.bfloat16,
                          kind="ExternalInput")
    # xt_d: k-chunk-major pack; wq_d: [v-all | q-all | k-all]
    wout_d = nc.dram_tensor("wout", [128, 4 * D], dt.bfloat16,
                            kind="ExternalInput")
    out_d = nc.dram_tensor("out", [T, D], dt.bfloat16, kind="ExternalOutput")
    cls_d = nc.dram_tensor("cls", [65, NH * F], dt.float32,
                           kind="ExternalOutput")
    vcls_d = nc.dram_tensor("vcls", [1, HD], dt.bfloat16,
                            kind="ExternalOutput")

    with tile.TileContext(nc) as tc, ExitStack() as ctx:
        # ---- static tiles (live for the whole kernel) ----
        stat = ctx.enter_context(tc.tile_pool(name="stat", bufs=1))
        xt_sb = stat.tile([128, KD, T], dt.bfloat16)
        wq_sb = stat.tile([128, KD, QKV], dt.bfloat16)
        wout_sb = stat.tile([128, 4, D], dt.bfloat16)
        qTx = stat.tile([128, 4, F * KB], dt.bfloat16)
        kTx = stat.tile([128, 4, F * KB + KPAD], dt.bfloat16)
        clsqk = stat.tile([128, 8], dt.float32)
        attnT = stat.tile([128, 4, T], dt.bfloat16)
        v_fr = stat.tile([128, 2 * F, NH, DH + 1], dt.bfloat16)
        vcls = stat.tile([1, NH, DH], dt.bfloat16)
        cls_st = stat.tile([128, NH, F], dt.float32)
        ones16 = stat.tile([128, 16], dt.float32)

        nc.vector.memset(ones16[:, :], 1.0)
        nc.vector.memset(kTx[:, :, F * KB:], 0.0)
        nc.vector.memset(v_fr[:, :, :, DH:DH + 1], 1.0)

        with ExitStack() as p1:
            # ---- input DMA ----
            # per-queue bandwidth is ~22GB/s, so parallelism needs many
            # queues; but each dispatch costs ~0.7-1us on its engine, so
            # spread dispatches over all three DMA-capable engines and size
            # transfers so arrival order matches the consumption order
            # (4 frame-aligned token waves x 8 contraction chunks)
            wq_v = wq_d[:, :].rearrange("p (s k q) -> p s k q", s=3, q=HD)
            dma_engs = [nc.sync, nc.scalar]
            di = [0]

            def in_dma(dst, src):
                dma_engs[di[0] % 2].dma_start(dst, src)
                di[0] += 1

            xt_v = xt_d[:, :].rearrange("p (k t) -> p k t", t=T)

            def xt_wave(r):
                t0 = 0 if r == 0 else 1 + 784 * r
                t1 = 1 + 784 * (r + 1)
                for k in range(KD):
                    in_dma(xt_sb[:, k, t0:t1], xt_v[:, k, t0:t1])

            for k in range(KD):
                in_dma(wq_sb[:, k, 0:HD], wq_v[:, 0, k, :])
            xt_wave(0)
            for k in range(KD):
                in_dma(wq_sb[:, k, HD:QKV], wq_v[:, 1:3, k, :])
            for r in range(1, 4):
                xt_wave(r)
            nc.sync.dma_start(
                wout_sb[:, 0:2, :],
                wout_d[:, 0:2 * D].rearrange("p (c o) -> p c o", o=D))
            nc.scalar.dma_start(
                wout_sb[:, 2:4, :],
                wout_d[:, 2 * D:].rearrange("p (c o) -> p c o", o=D))

            ps_gen = p1.enter_context(
                tc.tile_pool(name="ps_gen", bufs=3, space="PSUM"))
            ps_sim = p1.enter_context(
                tc.tile_pool(name="ps_sim", bufs=2, space="PSUM"))
            ps_ot = p1.enter_context(
                tc.tile_pool(name="ps_ot", bufs=3, space="PSUM"))
            sb_p = p1.enter_context(tc.tile_pool(name="sb_p", bufs=5))
            sb_r = p1.enter_context(tc.tile_pool(name="sb_r", bufs=2))
            sb_rb = p1.enter_context(tc.tile_pool(name="sb_rb", bufs=2))
            sb_o = p1.enter_context(tc.tile_pool(name="sb_o", bufs=2))

            qk_views = {}

            def qk_view(m):
                if m not in qk_views:
                    base = (qTx if m < 4 else kTx)[:, m % 4, 0:F * KB]
                    qk_views[m] = base.rearrange("p (f b) -> p f b", b=KB)
                return qk_views[m]

            def v_tile(f, jc):
                """project v for half-frame chunk (f, jc) into v_fr"""
                r0 = 1 + NP * f + 128 * jc
                rn = 128 if jc == 0 else NP - 128
                ps = ps_gen.tile([128, 512], dt.float32, tag="g")
                for k in range(KD):
                    nc.tensor.matmul(ps[:rn, :], xt_sb[:, k, r0:r0 + rn],
                                     wq_sb[:, k, 0:HD],
                                     start=(k == 0), stop=(k == KD - 1))
                psv = ps[:rn, :].rearrange("p (h d) -> p h d", d=DH)
                if jc == 0:
                    nc.scalar.copy(v_fr[:rn, 2 * f, :, 0:DH], psv)
                else:
                    nc.vector.tensor_copy(v_fr[:rn, 2 * f + 1, :, 0:DH], psv)

            def qk_chunk(m, c):
                """project q (m<4) / k (m>=4) slice m for frames 2c, 2c+1"""
                dst = qk_view(m)
                wc = HD + 128 * m if m < 4 else 2 * HD + 128 * (m - 4)
                t0 = 0 if c == 0 else 392 * c + 1
                nsz = 393 if c == 0 else 392
                ps = ps_gen.tile([128, 512], dt.float32, tag="g")
                for k in range(KD):
                    nc.tensor.matmul(
                        ps[:, :nsz], wq_sb[:, k, wc:wc + 128],
                        xt_sb[:, k, t0:t0 + nsz],
                        start=(k == 0), stop=(k == KD - 1))
                s0 = 1 if c == 0 else 0
                if c == 0:
                    nc.vector.tensor_copy(clsqk[:, m:m + 1], ps[:, 0:1])
                if m < 4:
                    nc.scalar.copy(dst[:, 2 * c:2 * c + 2, 0:NP],
                                   ps[:, s0:s0 + 2 * NP])
                else:
                    nc.vector.tensor_copy(dst[:, 2 * c:2 * c + 2, 0:NP],
                                          ps[:, s0:s0 + 2 * NP])

            def out_proj(t0, tn, split_dma=False):
                """project attnT cols [t0, t0+tn) through w_out"""
                o_sb = sb_o.tile([128, D], dt.bfloat16, tag="o")
                for half in range(2):
                    ps = ps_gen.tile([128, 512], dt.float32, tag="g")
                    for kc in range(4):
                        nc.tensor.matmul(
                            ps[:tn, :], attnT[:, kc, t0:t0 + tn],
                            wout_sb[:, kc, half * 512:(half + 1) * 512],
                            start=(kc == 0), stop=(kc == 3))
                    if half == 0:
                        nc.scalar.copy(o_sb[:tn, 0:512], ps[:tn, :])
                    else:
                        nc.vector.tensor_copy(o_sb[:tn, 512:1024], ps[:tn, :])
                if split_dma:
                    for rr in range(0, tn, 32):
                        rn = min(32, tn - rr)
                        nc.sync.dma_start(
                            out_d[1 + t0 + rr:1 + t0 + rr + rn, :],
                            o_sb[rr:rr + rn, :])
                else:
                    nc.sync.dma_start(out_d[1 + t0:1 + t0 + tn, :],
                                      o_sb[:tn, :])

            pend = {}

            def attn_sim(pr, f):
                """sim matmuls + exp for heads (2pr, 2pr+1) of frame f"""
                hc = pr
                fr0 = KB * f
                pTs = []
                for i in range(2):
                    pb = 64 * i
                    sim = ps_sim.tile([128, 2, KB], dt.float32, tag="sim")
                    nc.tensor.matmul(
                        sim[:, 0, :], kTx[pb:pb + 64, hc, fr0:fr0 + 128],
                        qTx[pb:pb + 64, hc, fr0:fr0 + KB],
                        start=True, stop=True)
                    nc.tensor.matmul(
                        sim[:, 1, :], kTx[pb:pb + 64, hc, fr0 + 128:fr0 + 256],
                        qTx[pb:pb + 64, hc, fr0:fr0 + KB],
                        start=True, stop=True)
                    pT = sb_p.tile([128, 2, KB], dt.bfloat16, tag="p")
                    nc.scalar.activation(pT[:, :, :], sim[:, :, :], AF.Exp)
                    pTs.append(pT)
                pend[(pr, f)] = pTs

            def attn_fin(pr, f):
                """ot matmuls + normalization for heads (2pr, 2pr+1)"""
                hc = pr
                pTs = pend.pop((pr, f))
                ots = []
                for i in range(2):
                    h = 2 * pr + i
                    pT = pTs[i]
                    # output + denominator via the v ones-column: rows 0..63
                    # = d, row 64 = sum(exp)
                    ot = ps_ot.tile([128, KB], dt.float32, tag="ot")
                    nc.tensor.matmul(
                        ot[0:65, :], v_fr[:, 2 * f, h, 0:DH + 1], pT[:, 0, :],
                        start=True, stop=False)
                    nc.tensor.matmul(
                        ot[0:65, :], v_fr[0:69, 2 * f + 1, h, 0:DH + 1],
                        pT[0:69, 1, :], start=False, stop=True)
                    # unnormalized cls numerator (+den at row 64) per frame;
                    # finalized on host
                    nc.scalar.copy(cls_st[0:65, h, f:f + 1], ot[0:65, NP:KB])
                    ots.append(ot)
                # normalize both heads: custom-DVE recip can't
                # partition-shift from PSUM, so stage via plain copies
                r2 = sb_r.tile([1, 2, NP], dt.float32, tag="r")
                r_t = sb_r.tile([1, 2, NP], dt.float32, tag="r2")
                rb = sb_rb.tile([128, 2, NP], dt.float32, tag="rb")
                for i in range(2):
                    nc.vector.tensor_copy(r2[0:1, i, :], ots[i][64:65, 0:NP])
                nc.vector.reciprocal_approx_fast(r_t[0:1, :, :], r2[0:1, :, :])
                nc.gpsimd.partition_broadcast(rb[:, :, :], r_t[0:1, :, :])
                for i in range(2):
                    dlo = 64 * i
                    nc.vector.tensor_mul(
                        attnT[dlo:dlo + 64, hc, NP * f:NP * (f + 1)],
                        ots[i][0:64, 0:NP], rb[dlo:dlo + 64, i, :])

            next_chunk = [0]

            def drain_out_proj(done_cols, reserve=0, limit=99):
                n = 0
                while ((next_chunk[0] + 1 + reserve) * 128 <= done_cols
                       and n < limit):
                    out_proj(128 * next_chunk[0], 128)
                    next_chunk[0] += 1
                    n += 1

            # ---- head: cls v-row, v for frames 0-3, qk chunk 0 ----
            ps = ps_gen.tile([128, 512], dt.float32, tag="g")
            for k in range(KD):
                nc.tensor.matmul(ps[:1, 0:HD], xt_sb[:, k, 0:1],
                                 wq_sb[:, k, 0:HD],
                                 start=(k == 0), stop=(k == KD - 1))
            nc.vector.tensor_copy(vcls[:, :, :], ps[:1, 0:HD])
            nc.sync.dma_start(vcls_d[:, :], vcls[:, :, :])
            # scatter cls_v into partition 68 of every odd v chunk (gpsimd
            # dispatch; idle until the first broadcasts at ~t=30us)
            for f in range(F):
                nc.gpsimd.dma_start(v_fr[68:69, 2 * f + 1, :, 0:DH],
                                    vcls[:, :, :])
            for f in range(4):
                v_tile(f, 0)
                v_tile(f, 1)

            # ---- frame-major main loop ----
            # fine interleave: each attn unit (whose ot/normalize chains run
            # on ACT/DVE/gpsimd) is sandwiched between qk-chunk, v-tile or
            # out-proj matmul groups so the in-order PE queue always has
            # independent work while the serial engines catch up. v for
            # frames 2c+4, 2c+5 is projected inside block c (needed at
            # block c+2 the earliest).
            for c in range(8):
                vf = 4 + 2 * c
                if c == 0:
                    for m in range(8):
                        qk_chunk(m, c)
                    # replicate cls q/k into column 196 of every frame block
                    # (cls lives in chunk 0, so one batched op per m)
                    for m in range(8):
                        nc.vector.tensor_scalar_mul(
                            qk_view(m)[:, :, NP], ones16[:, 0:F],
                            clsqk[:, m:m + 1])
                    for pr in range(4):
                        attn_sim(pr, 0)
                        if pr >= 2:
                            attn_fin(pr - 2, 0)
                        v_tile(vf + pr // 2, pr % 2)
                    attn_fin(2, 0)
                    attn_fin(3, 0)
                else:
                    fa, fb = 2 * c - 1, 2 * c
                    # qk slices for pair pr land right before pr's sims
                    for pr in range(4):
                        qk_chunk(pr, c)
                        qk_chunk(pr + 4, c)
                        attn_sim(pr, fa)
                        if pr == 1:
                            drain_out_proj(NP * fa, reserve=RESERVE, limit=1)
                        if pr >= 2:
                            attn_fin(pr - 2, fa)
                            attn_sim(pr - 2, fb)
                    attn_fin(2, fa)
                    attn_sim(2, fb)
                    if vf <= 14:
                        v_tile(vf, 0)
                        v_tile(vf, 1)
                    else:
                        drain_out_proj(NP * fa, reserve=RESERVE, limit=1)
                    attn_fin(3, fa)
                    attn_sim(3, fb)
                    attn_fin(0, fb)
                    if vf + 1 <= 15:
                        v_tile(vf + 1, 0)
                        v_tile(vf + 1, 1)
                    else:
                        drain_out_proj(NP * fa, reserve=RESERVE, limit=1)
                    attn_fin(1, fb)
                    attn_fin(2, fb)
                    drain_out_proj(NP * fa, reserve=RESERVE, limit=1)
                    attn_fin(3, fb)
                    drain_out_proj(NP * (fb + 1), reserve=RESERVE)
            # frame 15 interleaved with the reserved drain chunks
            attn_sim(0, 15)
            attn_sim(1, 15)
            drain_out_proj(NP * 15, reserve=2)
            attn_sim(2, 15)
            attn_sim(3, 15)
            drain_out_proj(NP * 15, reserve=1)
            attn_fin(0, 15)
            drain_out_proj(NP * 15, reserve=0)
            attn_fin(1, 15)
            attn_fin(2, 15)
            attn_fin(3, 15)
            drain_out_proj(NP * F)
            out_proj(TP // 128 * 128, TP - TP // 128 * 128, split_dma=True)
            # tiny cls exports; the cls output row is finalized on host
            nc.sync.dma_start(cls_d[:, :], cls_st[0:65, :, :])

    nc.compile()
    return nc


def _get_nc():
    if "nc" not in _CACHE:
        _CACHE["nc"] = _build_nc()
    return _CACHE["nc"]


def _pack_rows(a, kd):
    """[kd*128, N] -> [128, kd*N] with row p = concat_k a[128k+p]"""
    n = a.shape[1]
    return np.ascontiguousarray(
        a.reshape(kd, 128, n).transpose(1, 0, 2).reshape(128, kd * n))


def _prep_in_maps(x, w_qkv, w_out):
    x = np.asarray(x, dtype=np.float32)
    w_qkv = np.asarray(w_qkv, dtype=np.float32)
    w_out = np.asarray(w_out, dtype=np.float32)
    in_maps = []
    for core in range(N_CORES):
        b, hg = divmod(core, 2)
        cs = slice(hg * HD, (hg + 1) * HD)
        # wq_d: [v-all-k | q-all-k | k-all-k], each part [128, KD, 512]
        parts = [w_qkv[:, 2 * D:][:, cs], w_qkv[:, 0 * D:][:, cs] * SCALE,
                 w_qkv[:, 1 * D:][:, cs]]
        wq = np.concatenate([_pack_rows(p, KD) for p in parts], axis=1)
        wo = w_out[hg * HD:(hg + 1) * HD, :]
        in_maps.append({
            "xt": _pack_rows(np.ascontiguousarray(x[b].T), KD).astype(bf16),
            "wqkv": np.ascontiguousarray(wq).astype(bf16),
            "wout": _pack_rows(wo, 4).astype(bf16),
        })
    return in_maps


def _host_cls_row(res_core, w_out, hg, e0):
    """cls output row partial for one core from exported strips (fp32)."""
    cs = np.asarray(res_core["cls"], np.float32).reshape(65, NH, F)
    vq = np.asarray(res_core["vcls"], np.float32).reshape(NH, DH)
    num = cs[0:64].sum(-1)                       # [64, NH]
    den = cs[64].sum(-1)                         # [NH]
    num = num - (F - 1) * e0[None, :] * vq.T     # over-counted cls key
    den = den - (F - 1) * e0
    attnrow = (num / den[None, :]).T.reshape(HD)  # [NH*DH]
    return attnrow @ w_out[hg * HD:(hg + 1) * HD, :]


def run(x, w_qkv, w_out, trace=False):
    from concourse.bass_utils import run_bass_kernel_spmd

    nc = _get_nc()
    in_maps = _prep_in_maps(x, w_qkv, w_out)
    res = run_bass_kernel_spmd(nc, in_maps, list(range(N_CORES)), trace=trace)
    w_out32 = np.asarray(w_out, np.float32)
    # e0 = exp(q_cls . k_cls) per (batch, head), bf16-rounded like the device
    xb = np.asarray(x, np.float32)[:, 0, :].astype(bf16).astype(np.float32)
    wq8 = (np.asarray(w_qkv, np.float32)[:, 0:D] * SCALE
           ).astype(bf16).astype(np.float32)
    wk8 = np.asarray(w_qkv, np.float32)[:, D:2 * D
                                        ].astype(bf16).astype(np.float32)
    qc = (xb @ wq8).astype(bf16).astype(np.float32).reshape(B, 2 * NH, DH)
    kc = (xb @ wk8).astype(bf16).astype(np.float32).reshape(B, 2 * NH, DH)
    e0 = np.exp((qc * kc).sum(-1))                      # [B, 16]
    out = np.empty((B, T, D), dtype=np.float32)
    for b in range(B):
        out[b] = (np.asarray(res.results[2 * b]["out"], np.float32)
                  + np.asarray(res.results[2 * b + 1]["out"], np.float32))
        out[b, 0] = (
            _host_cls_row(res.results[2 * b], w_out32, 0, e0[b, 0:NH])
            + _host_cls_row(res.results[2 * b + 1], w_out32, 1, e0[b, NH:]))
    return out, res


def kernel(x, w_qkv, w_out, f):
    assert int(f) == F
    out, _ = run(x, w_qkv, w_out, trace=False)
    return out
